# revision 44
# baseline (speedup 1.0000x reference)
"""Trainium2 Bass kernel for the binarized-conv BasicBlock problem.

Math restructure (exact up to fp16-grade rounding):
  wb = sign(weight)  (+-1 exactly representable in fp16)
  out = clip( A * conv(x, wb~) + B , -1, 1 )
where
  A[o]     = gamma/sqrt(var+eps) * (1 + w1[o])                (per channel)
  B[o,h,w] = bs*(conv(S1,wb) + w1*conv(S2,wb))[o,h,w] + bb[o] (batch-independent
             shift/edge field, computed on host)
  wb~      = wb + diag(1/A) on the center tap, so the conv also carries the
             residual:  A*(conv(x,wb) + x/A) = A*conv(x,wb) + x.
Precision: x is split on host as x = x16 + d16 (both fp16), so the conv is
fp32-accurate: conv(x) = conv(x16) + conv(d16) with near-exact +-1 weights
contracted as K=128 = [x16 ch; d16 ch] in one matmul stream per image.
Measured end-to-end max err ~3e-3 against the fp32 reference (threshold 2e-2).

PE mapping: per image pair, rhsA = [x16 A; d16 A], rhsB = [d16 B; x16 B].
Each tap is two K=128, M=64 matmuls on the two column halves of the PE
array; redundant LDWEIGHTS are deduplicated post-legalize so the column
halves stream concurrently with weights held stationary across the row
chunks of a tap.  The 7 PSUM row-chunks are processed in two groups
(chunks 0-3, then 4-6): each group's 2-op DVE epilogue
  u16 = fp16(A*psum + B16); o16 = clip(u16,-1,1)  -> fp16 DMA out
drains while the other group's matmuls stream, so the PE never stalls on
PSUM reuse.  Input images are DMA'd in two row-bands matching the groups.
Sharding: batch 64 -> 8 cores x 8 images.
"""
import sys
for _p in ('/opt/trn_rl_repo',):
    if _p not in sys.path:
        sys.path.insert(0, _p)

import numpy as np
import concourse.bass as bass
import concourse.bacc as bacc
import concourse.tile as tile
import concourse.mybir as mybir
from concourse import bass_utils

BN_EPS = 1e-5
N_CORES = 8
C, H, W = 64, 56, 56
HP, WP = H + 2, W + 2           # padded spatial
NPOS = H * W                    # 3136
PPOS = HP * WP                  # 3364
IMGS = 8                        # images per core
PAIRS = IMGS // 2
CH_ROWS = 8                     # output rows per chunk
NCHUNK = H // CH_ROWS           # 7
CHL = CH_ROWS * W               # 448
GROUPS = ((0, 1, 2, 3), (4, 5, 6))
# padded-row bands backing each group (with 3x3 halo):
#   group 0 reads padded rows 0..33 ; group 1 reads padded rows 32..57
G0_ROWS = 34
G1_BASE = 32                    # first padded row of band 1
G1_ROWS = HP - G1_BASE          # 26
WARMUP_MMS = 26                 # dummy matmuls during DMA fill to trip HAM warm

f32 = mybir.dt.float32
f16 = mybir.dt.float16
ALU = mybir.AluOpType

_CACHE = {}


def _ldw_sig(inst):
    ap = inst.ins[0]
    bap = ap.bass_ap
    return (bap.tensor.name, bap.offset, str(bap.ap), str(inst.tile_position))


def _dedup_ldweights(ordered, enable=True):
    """Drop LDWEIGHTS that reload the exact weights already resident in the
    same PE column group, so matmuls on alternating column halves stream
    back-to-back (a reload between them serializes the halves: the load
    conflicts with the in-flight matmul's row groups)."""
    if not enable:
        return ordered
    for bb, insts in ordered.items():
        last = {}     # col position -> (sig, kept name)
        remap = {}
        keep = []
        pending = None  # dropped ldw awaiting its matmul to absorb deps
        for inst in insts:
            if isinstance(inst, mybir.InstLdweights):
                sig = _ldw_sig(inst)
                col = (inst.tile_position or (0, 0))[1]
                ent = last.get(col)
                if ent is not None and ent[0] == sig:
                    remap[inst.name] = ent[1]
                    pending = inst
                    continue
                last[col] = (sig, inst.name)
            elif isinstance(inst, mybir.InstMatmult) and pending is not None:
                try:
                    inst.merge_dependencies_from(pending)
                except Exception:
                    inst.add_sync_dependencies_from(pending)
                pending = None
            keep.append(inst)
        if remap:
            for inst in keep:
                inst.remap_dependency_names(remap)
        ordered[bb] = keep
    return ordered


def _build_module(repeat=1, ablate=(), compile=True, dedup=True,
                  warmup=WARMUP_MMS):
    nc = bacc.Bacc("TRN2", target_bir_lowering=False, debug=False,
                   enable_asserts=False, num_devices=N_CORES)

    # pre-padded on host: [pair, {A,B}, 128, 58*58] fp16
    xr_d = nc.dram_tensor("xr", [PAIRS, 2, 128, HP, WP], f16, kind="ExternalInput").ap()
    w_d = nc.dram_tensor("wt", [128, 9 * 128], f16, kind="ExternalInput").ap()
    a_d = nc.dram_tensor("ascale", [128, 1], f32, kind="ExternalInput").ap()
    b_d = nc.dram_tensor("bfield", [128, NPOS], f16, kind="ExternalInput").ap()
    y_d = nc.dram_tensor("y", [PAIRS, 128, NPOS], f16, kind="ExternalOutput").ap()

    import concourse.tile as tile_mod
    orig_legalize = tile_mod.tile_legalize
    tile_mod.tile_legalize = lambda ordered, nc_, _o=orig_legalize: _dedup_ldweights(
        _o(ordered, nc_), enable=dedup)
    try:
        with tile.TileContext(nc) as tc:
            with tc.tile_pool(name="const", bufs=1) as constp, \
                 tc.tile_pool(name="rhs", bufs=2) as rhsp, \
                 tc.tile_pool(name="eout", bufs=4) as outp, \
                 tc.tile_pool(name="psum", bufs=1, space="PSUM") as psp:
                # The weight tile is split (tap 0 / taps 1-2 / taps 3-8) so
                # early LDWEIGHTS wait on small transfers instead of the
                # full 295KB load: DMA engines round-robin all in-flight
                # transfers, so every byte ahead of a dependency delays it.
                # Input DMAs are issued from the (otherwise idle) Scalar
                # queue: the Sync queue pays its table load + barrier
                # later, and carries the output DMAs instead.
                wtA = constp.tile([128, 128], f16)
                nc.scalar.dma_start(wtA[:], w_d[:, 0:128])
                wtC = constp.tile([128, 2 * 128], f16)
                wtB = constp.tile([128, 6 * 128], f16)
                at = constp.tile([128, 1], f32)
                bt = constp.tile([128, NPOS], f16)

                # PE warmup during the DMA fill: dummy matmuls on a zeroed
                # tile (no DMA dependency) keep the HAM activity window
                # busy so real matmuls start at full clock.  Sized to
                # finish as the first input band lands.
                if warmup:
                    wrm = constp.tile([128, 256], f16)
                    nc.gpsimd.memset(wrm[:], 0.0)
                    wps = psp.tile([64, 256], f32, tag="wps", name="wps")
                    for i in range(warmup):
                        nc.tensor.matmul(wps[:], wrm[:, 0:64], wrm[:],
                                         start=True, stop=True,
                                         tile_position=(0, 0),
                                         skip_group_check=True)

                def _body():
                  first = True
                  plist = [pp for _ in range(repeat) for pp in range(PAIRS)]
                  for pi, p in enumerate(plist):
                    is_last_pair = pi == len(plist) - 1
                    # two row-bands per image, matching the chunk groups
                    rT = [rhsp.tile([128, G0_ROWS, WP], f16, tag=f"r{j}T",
                                    name=f"r{j}T")
                          for j in range(2)]
                    rB = [rhsp.tile([128, G1_ROWS, WP], f16, tag=f"r{j}B",
                                    name=f"r{j}B")
                          for j in range(2)]
                    if 'dma_in' not in ablate:
                        # All input DMAs on the Scalar queue, strictly in
                        # criticality order: img-A band, taps-1-2 weights
                        # (which the h0 stream consumes while img-B's band
                        # is still in flight), img-B band, the rest.
                        nc.scalar.dma_start(rT[0][:], xr_d[p, 0, :, 0:G0_ROWS])
                        if first:
                            nc.scalar.dma_start(wtC[:], w_d[:, 128:3 * 128])
                        nc.scalar.dma_start(rT[1][:], xr_d[p, 1, :, 0:G0_ROWS])
                        if first:
                            # taps 3-8 weights: needed ~4.5us after the
                            # first matmul.
                            nc.scalar.dma_start(wtB[:], w_d[:, 3 * 128:9 * 128])
                        for j in range(2):
                            nc.scalar.dma_start(rB[j][:], xr_d[p, j, :, G1_BASE:HP])
                    if first:
                        # constants land after pair-0 input; they are only
                        # needed by the first epilogue (~14us in)
                        nc.scalar.dma_start(at[:], a_d[:])
                        nc.scalar.dma_start(bt[:], b_d[:])
                        first = False

                    pss = {}
                    for c in range(NCHUNK):
                        pss[c] = psp.tile([128, CHL], f32, tag=f"ps{c}", name=f"ps{c}")

                    def _band(c):
                        if c < 4:
                            return rT, 0
                        return rB, G1_BASE

                    # last pair: finish on a 1-chunk group so the final
                    # epilogue+DMA tail is ~1us instead of ~2.6us
                    groups = GROUPS if not is_last_pair else \
                        ((0, 1, 2, 3), (4, 5), (6,))
                    for gi, chunks in enumerate(groups):
                        if 'matmul' in ablate:
                            for c in chunks:
                                nc.vector.tensor_copy(pss[c][:], bt[:, c * CHL:(c + 1) * CHL])
                        else:
                            # pair 0, first group: the img-A (h0) column
                            # half runs taps 0-2 alone while img-B's input
                            # band is still in flight, then img-B catches
                            # up -- hides the second DMA entirely.  Within
                            # the solo phase, start=True (PSUM-clearing)
                            # matmuls are interleaved with tap-1 matmuls:
                            # consecutive same-column start matmuls pay a
                            # ~2x issue cadence.
                            if pi == 0 and gi == 0:
                                solo = ([(t, c) for c in chunks for t in (0, 1)]
                                        + [(2, c) for c in chunks])
                                order = ([(t, c, 0) for t, c in solo]
                                         + [(t, c, 1) for t, c in solo]
                                         + [(t, c, j) for t in range(3, 9)
                                            for c in chunks for j in range(2)])
                            else:
                                order = [(t, c, j) for t in range(9)
                                         for c in chunks for j in range(2)]
                            for t, c, j in order:
                                k, l = divmod(t, 3)
                                if t == 0:
                                    wsrc, toff = wtA, 0
                                elif t < 3:
                                    wsrc, toff = wtC, t - 1
                                else:
                                    wsrc, toff = wtB, t - 3
                                band, rbase = _band(c)
                                r0 = CH_ROWS * c + k - rbase
                                rhs = band[j][:, r0: r0 + CH_ROWS, l: l + W]
                                lhsT = wsrc[:, toff * 128 + 64 * j: toff * 128 + 64 * j + 64]
                                out_ap = pss[c][64 * j: 64 * j + 64, :]
                                nc.tensor.matmul(out_ap, lhsT, rhs,
                                                 start=(t == 0), stop=(t == 8),
                                                 tile_position=(0, 64 * j),
                                                 skip_group_check=True)

                        # group output tile: clamp results for all the
                        # group's chunks accumulate here, then ship in ONE
                        # DMA (a per-chunk DMA costs ~700ns of issue time
                        # on the Sync queue each).  The very last group
                        # ships per-chunk instead so the final transfer is
                        # small and the teardown starts earlier.
                        glen = len(chunks)
                        per_chunk_out = is_last_pair and gi == len(groups) - 1
                        og = outp.tile([128, glen * CHL], f16, tag=f"og{gi}",
                                       name=f"og{gi}")
                        for ci, c in enumerate(chunks):
                            ps = pss[c]
                            osl = og[:, ci * CHL:(ci + 1) * CHL]
                            if 'epilogue' in ablate:
                                nc.vector.tensor_copy(osl, ps[:])
                            else:
                                # u16 = fp16(A*psum + B); o16 = clip(u16,-1,1)
                                u = outp.tile([128, CHL], f16, tag="u")
                                nc.vector.scalar_tensor_tensor(
                                    u[:], ps[:], at[:], bt[:, c * CHL:(c + 1) * CHL],
                                    ALU.mult, ALU.add)
                                nc.vector.tensor_scalar(osl, u[:], 1.0, -1.0,
                                                        ALU.min, ALU.max)
                            if per_chunk_out and 'dma_out' not in ablate:
                                nc.sync.dma_start(
                                    y_d[p][:, c * CHL:(c + 1) * CHL], osl)
                        if not per_chunk_out and 'dma_out' not in ablate:
                            c0 = chunks[0]
                            nc.sync.dma_start(
                                y_d[p][:, c0 * CHL:(c0 + glen) * CHL], og[:])

                _body()
    finally:
        tile_mod.tile_legalize = orig_legalize

    if compile:
        nc.compile()
    return nc


def _host_prep(x, shift1, shift2, weight, w1, gamma, beta, running_mean, running_var):
    x = np.asarray(x, np.float32)
    s1 = np.asarray(shift1, np.float32).reshape(C)
    s2 = np.asarray(shift2, np.float32).reshape(C)
    w = np.asarray(weight, np.float32)
    w1v = np.asarray(w1, np.float32).reshape(C)
    gamma = np.asarray(gamma, np.float32)
    beta = np.asarray(beta, np.float32)
    mean = np.asarray(running_mean, np.float32)
    var = np.asarray(running_var, np.float32)

    wb = np.sign(w).astype(np.float32)
    bs = (gamma / np.sqrt(var + BN_EPS)).astype(np.float32)
    A = (bs * (1.0 + w1v)).astype(np.float32)
    bb = (beta - mean * bs).astype(np.float32)
    invA = (1.0 / A).astype(np.float32)

    G1 = np.einsum('oikl,i->okl', wb, s1)
    G2 = np.einsum('oikl,i->okl', wb, s2)
    G = bs[:, None, None] * (G1 + w1v[:, None, None] * G2)
    B = np.zeros((C, H, W), np.float32)
    hh = np.arange(H)[:, None]
    ww = np.arange(W)[None, :]
    for k in range(3):
        for l in range(3):
            m = ((hh + k - 1 >= 0) & (hh + k - 1 < H) &
                 (ww + l - 1 >= 0) & (ww + l - 1 < W)).astype(np.float32)
            B += G[:, k, l][:, None, None] * m[None]
    B += bb[:, None, None]

    # weights: lhsT[k, m] = wb[m, k, t].  Per tap t:
    #   cols 0-63  (img A): rows 0-63 = x16 wts, rows 64-127 = d16 wts
    #   cols 64-127(img B): rows 0-63 = d16 wts, rows 64-127 = x16 wts
    # Center tap carries diag(1/A) on BOTH the x16 and d16 rows so the
    # matmul output includes (x16+d16)/A and the BN scale restores +x.
    wbT = wb.transpose(1, 0, 2, 3)  # [i, o, k, l]
    wtile = np.zeros((128, 9 * 128), np.float32)
    identA = np.diag(invA)
    for t in range(9):
        k, l = divmod(t, 3)
        blk = wbT[:, :, k, l]  # [i(K), o(M)]
        ident = identA if t == 4 else 0.0
        wtile[0:64, t * 128: t * 128 + 64] = blk + ident         # img A x16
        wtile[64:128, t * 128: t * 128 + 64] = blk + ident       # img A d16
        wtile[0:64, t * 128 + 64: t * 128 + 128] = blk + ident   # img B d16
        wtile[64:128, t * 128 + 64: t * 128 + 128] = blk + ident # img B x16
    wtile16 = wtile.astype(np.float16)

    x16 = x.astype(np.float16)
    d16 = (x - x16.astype(np.float32)).astype(np.float16)

    N = x.shape[0]
    xr = np.zeros((N // 2, 2, 128, HP, WP), np.float16)
    # rhsA = [x16 imgA; d16 imgA]; rhsB = [d16 imgB; x16 imgB] (flipped)
    xr[:, 0, 0:64, 1:H + 1, 1:W + 1] = x16[0::2]
    xr[:, 0, 64:128, 1:H + 1, 1:W + 1] = d16[0::2]
    xr[:, 1, 0:64, 1:H + 1, 1:W + 1] = d16[1::2]
    xr[:, 1, 64:128, 1:H + 1, 1:W + 1] = x16[1::2]

    a128 = np.concatenate([A, A]).reshape(128, 1).astype(np.float32)
    b128 = np.concatenate([B.reshape(C, NPOS)] * 2, axis=0).astype(np.float16)
    return xr, wtile16, a128, b128


def kernel(**inputs):
    xr, wtile16, a128, b128 = _host_prep(**inputs)
    if 'nc' not in _CACHE:
        _CACHE['nc'] = _build_module()
    nc = _CACHE['nc']

    in_maps = []
    for core in range(N_CORES):
        in_maps.append({
            "xr": np.ascontiguousarray(xr[core * PAIRS:(core + 1) * PAIRS]),
            "wt": wtile16,
            "ascale": a128,
            "bfield": b128,
        })
    _CACHE['in_maps'] = in_maps
    res = bass_utils.run_bass_kernel_spmd(nc, in_maps,
                                          core_ids=list(range(N_CORES)))
    _CACHE['last_result'] = res

    N = N_CORES * IMGS
    y = np.empty((N, C, H, W), np.float32)
    for core in range(N_CORES):
        yc = res.results[core]["y"]  # [PAIRS, 128, NPOS] fp16
        yc = yc.astype(np.float32).reshape(PAIRS * 2, C, H, W)
        y[core * IMGS:(core + 1) * IMGS] = yc
    return y
